# revision 12
# baseline (speedup 1.0000x reference)
"""Trainium2 Bass kernel for EnhancedGRU memory updater (scatter_memory).

kernel(**inputs) takes FULL inputs, shards the memory table row-wise over 8
NeuronCores, routes messages to owner shards (ids are sorted so routing is
slicing), and per shard runs on-device:
  - bulk DRAM->DRAM copy of the shard into the output table
  - dma_gather of the active rows (int16 indices relative to 2 sub-shards)
  - GRU cell + 2 linear layers in [gate, u] layout (fp32r matmuls)
  - dma_scatter_add of masked row deltas (new - old) onto the copied table
last_update is assembled host-side (0.4% of total I/O).

Self-contained: hardcodes shapes from the problem spec.
"""

import os
import numpy as np

N_NODES = 500_000
MEM_DIM = 256
MSG_DIM = 172
N_UNIQUE = 131_072
FC_DIM = 64
N_CORES = 8
R = N_NODES // N_CORES          # rows per shard = 62500
HALF = R // 2                    # sub-shard rows (int16-indexable)
P = 128
CHUNK = 512                      # u rows per compute chunk
BLK = 512                        # u rows per gather/scatter DMA op (== CHUNK)

_CACHE = {}


def _build(u_sub):
    import concourse.bass as bass
    import concourse.bacc as bacc
    import concourse.mybir as mybir
    import concourse.tile as tile
    from concourse.bass import _add_dep_helper
    from concourse.masks import make_identity

    f32 = mybir.dt.float32
    f32r = mybir.dt.float32r
    i16 = mybir.dt.int16
    AF = mybir.ActivationFunctionType
    OP = mybir.AluOpType

    u_tot = 2 * u_sub
    n_tiles = u_tot // P
    n_chunks = u_tot // CHUNK
    n_blocks = u_tot // BLK
    blocks_per_sub = u_sub // BLK
    icols = BLK // 16                # idx columns per block

    nc = bacc.Bacc(None, target_bir_lowering=False, debug=False)

    tab_in = nc.dram_tensor("tab_in", (R, MEM_DIM), f32, kind="ExternalInput")
    msgsT = nc.dram_tensor("msgsT", (MSG_DIM, u_tot), f32r, kind="ExternalInput")
    idx16 = nc.dram_tensor("idx16", (P, u_tot // 16), i16, kind="ExternalInput")
    mask = nc.dram_tensor("mask", (P, n_tiles), f32, kind="ExternalInput")
    w_ihT = nc.dram_tensor("w_ihT", (MSG_DIM, 3 * MEM_DIM), f32r, kind="ExternalInput")
    w_hhT = nc.dram_tensor("w_hhT", (MEM_DIM, 3 * MEM_DIM), f32r, kind="ExternalInput")
    w_fcT = nc.dram_tensor("w_fcT", (MEM_DIM, FC_DIM), f32r, kind="ExternalInput")
    w_outT = nc.dram_tensor("w_outT", (FC_DIM + 1, MEM_DIM), f32r, kind="ExternalInput")
    biases = nc.dram_tensor("biases", (P, 9), f32, kind="ExternalInput")

    tab_out = nc.dram_tensor("tab_out", (R, MEM_DIM), f32, kind="ExternalOutput")

    with tile.TileContext(nc) as tc:
        with tc.tile_pool(name="const", bufs=1) as cp, \
             tc.tile_pool(name="store", bufs=1) as sp, \
             tc.tile_pool(name="work", bufs=2) as wp, \
             tc.tile_pool(name="gate", bufs=1) as gp, \
             tc.tile_pool(name="ps", bufs=2, space="PSUM") as pp, \
             tc.tile_pool(name="psrz", bufs=4, space="PSUM") as prz, \
             tc.tile_pool(name="pstail", bufs=2, space="PSUM") as ptl:

            # ---- big shard copy: input -> output, 16 flat ~4MB DMAs on SP
            flat_in = tab_in[:].rearrange("r d -> (r d)")
            flat_out = tab_out[:].rearrange("r d -> (r d)")
            tot = R * MEM_DIM
            ncopy = 16
            cs = tot // ncopy
            assert cs * ncopy == tot
            copies = []
            for j in range(ncopy):
                copies.append(nc.sync.dma_start(
                    out=flat_out[j * cs:(j + 1) * cs],
                    in_=flat_in[j * cs:(j + 1) * cs]))

            # ---- persistent constants (ACT HWDGE queue)
            wih0 = cp.tile([P, 3 * MEM_DIM], f32r, tag="wih0")
            wih1 = cp.tile([MSG_DIM - P, 3 * MEM_DIM], f32r, tag="wih1")
            whh0 = cp.tile([P, 3 * MEM_DIM], f32r, tag="whh0")
            whh1 = cp.tile([P, 3 * MEM_DIM], f32r, tag="whh1")
            wfc0 = cp.tile([P, FC_DIM], f32r, tag="wfc0")
            wfc1 = cp.tile([P, FC_DIM], f32r, tag="wfc1")
            wout = cp.tile([FC_DIM + 1, MEM_DIM], f32r, tag="wout")
            bias_t = cp.tile([P, 9], f32, tag="bias")
            idx_t = cp.tile([P, u_tot // 16], i16, tag="idx")
            mask_t = cp.tile([P, n_tiles], f32, tag="mask")
            ident = cp.tile([P, P], f32, tag="ident")

            nc.scalar.dma_start(out=wih0[:], in_=w_ihT[0:P, :])
            nc.scalar.dma_start(out=wih1[:], in_=w_ihT[P:MSG_DIM, :])
            nc.scalar.dma_start(out=whh0[:], in_=w_hhT[0:P, :])
            nc.scalar.dma_start(out=whh1[:], in_=w_hhT[P:2 * P, :])
            nc.scalar.dma_start(out=wfc0[:], in_=w_fcT[0:P, :])
            nc.scalar.dma_start(out=wfc1[:], in_=w_fcT[P:2 * P, :])
            nc.scalar.dma_start(out=wout[:], in_=w_outT[:])
            nc.scalar.dma_start(out=bias_t[:], in_=biases[:])
            nc.scalar.dma_start(out=idx_t[:], in_=idx16[:])
            nc.scalar.dma_start(out=mask_t[:], in_=mask[:])
            make_identity(nc, ident[:])

            # ---- persistent delta store: [128, n_tiles * 256] f32
            out_store = sp.tile([P, n_tiles * MEM_DIM], f32, tag="ostore")
            os3 = out_store[:].rearrange("p (t d) -> p t d", d=MEM_DIM)

            for b in range(n_blocks):
                sub = b // blocks_per_sub
                # gather BLK h rows (relative to sub-shard): [128, 4, 256]
                g_blk = wp.tile([P, BLK // P, MEM_DIM], f32, tag="g")
                nc.gpsimd.dma_gather(
                    out_ap=g_blk[:],
                    in_ap=tab_in[sub * HALF:(sub + 1) * HALF, :],
                    idxs_ap=idx_t[:, b * icols:(b + 1) * icols],
                    num_idxs=BLK, num_idxs_reg=BLK, elem_size=MEM_DIM)

                if True:
                    c = b
                    u0 = c * CHUNK
                    xT0 = wp.tile([P, CHUNK], f32r, tag="xT0")
                    xT1 = wp.tile([MSG_DIM - P, CHUNK], f32r, tag="xT1")
                    nc.scalar.dma_start(out=xT0[:], in_=msgsT[0:P, u0:u0 + CHUNK])
                    nc.scalar.dma_start(out=xT1[:],
                                        in_=msgsT[P:MSG_DIM, u0:u0 + CHUNK])

                    def hsl(k, g_blk=g_blk):
                        return g_blk[:, k, :]

                    # transpose h -> hT (2 x [128, 512]) via PE into PSUM
                    hTp0 = pp.tile([P, CHUNK], f32, tag="pA", space="PSUM")
                    hTp1 = pp.tile([P, CHUNK], f32, tag="pA", space="PSUM")
                    for k in range(4):
                        nc.tensor.transpose(out=hTp0[:, k * P:(k + 1) * P],
                                            in_=hsl(k)[:, 0:P], identity=ident[:])
                        nc.tensor.transpose(out=hTp1[:, k * P:(k + 1) * P],
                                            in_=hsl(k)[:, P:MEM_DIM],
                                            identity=ident[:])
                    hT0 = gp.tile([P, CHUNK], f32r, tag="hT0")
                    hT1 = gp.tile([P, CHUNK], f32r, tag="hT1")
                    nc.vector.tensor_copy(hT0[:], hTp0[:])
                    nc.vector.tensor_copy(hT1[:], hTp1[:])
                    hT_sb = [hT0, hT1]

                    # gh_n = W_hh[512:768] @ hT -> 2 psum tiles [128, 512]
                    ghn_ps = []
                    for g2 in range(2):
                        gs = 2 * MEM_DIM + g2 * P
                        t = pp.tile([P, CHUNK], f32, tag="pA", space="PSUM")
                        ghn_ps.append(t)
                        nc.tensor.matmul(out=t[:], lhsT=whh0[:, gs:gs + P],
                                         rhs=hT0[:], start=True, stop=False)
                        nc.tensor.matmul(out=t[:], lhsT=whh1[:, gs:gs + P],
                                         rhs=hT1[:], start=False, stop=True)
                    # r,z: gi+gh accumulated, gate rows 0..511 (4 psum tiles)
                    rz_ps = []
                    for g4 in range(4):
                        gs = g4 * P
                        t = prz.tile([P, CHUNK], f32, tag="rz", space="PSUM")
                        rz_ps.append(t)
                        nc.tensor.matmul(out=t[:], lhsT=wih0[:, gs:gs + P],
                                         rhs=xT0[:], start=True, stop=False)
                        nc.tensor.matmul(out=t[:], lhsT=wih1[:, gs:gs + P],
                                         rhs=xT1[:], start=False, stop=False)
                        nc.tensor.matmul(out=t[:], lhsT=whh0[:, gs:gs + P],
                                         rhs=hT0[:], start=False, stop=False)
                        nc.tensor.matmul(out=t[:], lhsT=whh1[:, gs:gs + P],
                                         rhs=hT1[:], start=False, stop=True)
                    # ghn + b_hhn -> SBUF (ACT, bias cols 4,5)
                    ghn_sb = []
                    for g2 in range(2):
                        t = gp.tile([P, CHUNK], f32, tag=f"ghnb{g2}")
                        nc.scalar.activation(t[:], ghn_ps[g2][:], AF.Identity,
                                             bias=bias_t[:, 4 + g2:5 + g2])
                        ghn_sb.append(t)
                    # gi_n = W_ih[512:768] @ xT -> reuse pA psum slots
                    gin_ps = []
                    for g2 in range(2):
                        gs = 2 * MEM_DIM + g2 * P
                        t = pp.tile([P, CHUNK], f32, tag="pA", space="PSUM")
                        gin_ps.append(t)
                        nc.tensor.matmul(out=t[:], lhsT=wih0[:, gs:gs + P],
                                         rhs=xT0[:], start=True, stop=False)
                        nc.tensor.matmul(out=t[:], lhsT=wih1[:, gs:gs + P],
                                         rhs=xT1[:], start=False, stop=True)
                    # r,z = sigmoid(rz + b) (bias cols 0..3)
                    rz_sb = []
                    for g4 in range(4):
                        t = gp.tile([P, CHUNK], f32, tag=f"rz{g4}")
                        nc.scalar.activation(t[:], rz_ps[g4][:], AF.Sigmoid,
                                             bias=bias_t[:, g4:g4 + 1])
                        rz_sb.append(t)
                    r_sb, z_sb = rz_sb[0:2], rz_sb[2:4]
                    # n = tanh(gin + r*ghn_b + b_ihn) (bias cols 6,7)
                    hn_sb = []
                    for g2 in range(2):
                        tm = gp.tile([P, CHUNK], f32, tag=f"tm{g2}")
                        nc.vector.tensor_tensor(out=tm[:], in0=r_sb[g2][:],
                                                in1=ghn_sb[g2][:], op=OP.mult)
                        nc.vector.tensor_tensor(out=tm[:], in0=tm[:],
                                                in1=gin_ps[g2][:], op=OP.add)
                        n_t = gp.tile([P, CHUNK], f32, tag=f"n{g2}")
                        nc.scalar.activation(n_t[:], tm[:], AF.Tanh,
                                             bias=bias_t[:, 6 + g2:7 + g2])
                        # h_new = n + z*(h - n); d reuses the tm slot,
                        # h_new reuses the hT slot (both dead by then)
                        d_t = gp.tile([P, CHUNK], f32, tag=f"tm{g2}")
                        nc.vector.tensor_tensor(out=d_t[:], in0=hT_sb[g2][:],
                                                in1=n_t[:], op=OP.subtract)
                        nc.vector.tensor_tensor(out=d_t[:], in0=z_sb[g2][:],
                                                in1=d_t[:], op=OP.mult)
                        hn = gp.tile([P, CHUNK], f32r, tag=f"hT{g2}")
                        nc.vector.tensor_tensor(out=hn[:], in0=n_t[:],
                                                in1=d_t[:], op=OP.add)
                        hn_sb.append(hn)
                    # predT = W_fc @ h_new (+b_fc); augmented with a ones row
                    pred_ps = ptl.tile([FC_DIM, CHUNK], f32, tag="tail",
                                       space="PSUM")
                    nc.tensor.matmul(out=pred_ps[:], lhsT=wfc0[:, :],
                                     rhs=hn_sb[0][:], start=True, stop=False)
                    nc.tensor.matmul(out=pred_ps[:], lhsT=wfc1[:, :],
                                     rhs=hn_sb[1][:], start=False, stop=True)
                    predT = gp.tile([FC_DIM + 1, CHUNK], f32r, tag="predT")
                    nc.scalar.activation(predT[0:FC_DIM, :], pred_ps[:],
                                         AF.Identity, bias=bias_t[0:FC_DIM, 8:9])
                    nc.scalar.activation(predT[FC_DIM:FC_DIM + 1, :],
                                         pred_ps[0:1, :], AF.Copy,
                                         bias=1.0, scale=0.0)
                    # out rows (+b_out via ones row), delta = mask*(out - h)
                    for k in range(4):
                        ops = ptl.tile([P, MEM_DIM], f32, tag="tail",
                                       space="PSUM")
                        nc.tensor.matmul(out=ops[:],
                                         lhsT=predT[:, k * P:(k + 1) * P],
                                         rhs=wout[:], start=True, stop=True)
                        t_u = 4 * c + k
                        dl = gp.tile([P, MEM_DIM], f32, tag="dl")
                        nc.vector.tensor_tensor(out=dl[:], in0=ops[:],
                                                in1=hsl(k)[:], op=OP.subtract)
                        nc.vector.tensor_scalar_mul(
                            os3[:, t_u, :], dl[:], mask_t[:, t_u:t_u + 1])

            # ---- scatter-add masked deltas onto the copied table
            for b in range(n_blocks):
                sub = b // blocks_per_sub
                sa = nc.gpsimd.dma_scatter_add(
                    out_ap=tab_out[sub * HALF:(sub + 1) * HALF, :],
                    in_ap=os3[:, b * (BLK // P):(b + 1) * (BLK // P), :],
                    idxs_ap=idx_t[:, b * icols:(b + 1) * icols],
                    num_idxs=BLK, num_idxs_reg=BLK, elem_size=MEM_DIM)
                for cpi in copies:
                    _add_dep_helper(sa.ins, cpi.ins, sync=True,
                                    reason="scatter-add after table copy")

    nc.compile()
    return nc


def _prep(inputs):
    """Host-side shard/routing prep. Returns (u_sub, in_maps, (ids, ts))."""
    ids = np.asarray(inputs["unique_node_ids"])
    msgs = np.asarray(inputs["unique_messages"], dtype=np.float32)
    ts = np.asarray(inputs["timestamps"], dtype=np.float32)
    tab = np.asarray(inputs["memory_table"], dtype=np.float32)

    if not np.all(ids[:-1] <= ids[1:]):
        order = np.argsort(ids, kind="stable")
        ids, msgs, ts = ids[order], msgs[order], ts[order]

    bounds = np.searchsorted(ids, np.arange(N_CORES + 1) * R).astype(np.int64)
    mids = np.searchsorted(ids, np.arange(N_CORES) * R + HALF).astype(np.int64)
    sub_cnt = np.stack([mids - bounds[:-1], bounds[1:] - mids])  # [2, C]
    u_sub = int(np.ceil(max(int(sub_cnt.max()), BLK) / BLK) * BLK)
    u_tot = 2 * u_sub
    n_tiles = u_tot // P

    w_ihT = np.ascontiguousarray(np.asarray(inputs["W_ih"], np.float32).T)
    w_hhT = np.ascontiguousarray(np.asarray(inputs["W_hh"], np.float32).T)
    w_fcT = np.ascontiguousarray(np.asarray(inputs["W_fc"], np.float32).T)
    w_out = np.asarray(inputs["W_out"], np.float32)
    b_out = np.asarray(inputs["b_out"], np.float32)
    w_outT = np.ascontiguousarray(
        np.concatenate([w_out.T, b_out[None, :]], axis=0))  # [65, 256]
    b_ih = np.asarray(inputs["b_ih"], np.float32)
    b_hh = np.asarray(inputs["b_hh"], np.float32)
    b_fc = np.asarray(inputs["b_fc"], np.float32)

    biases = np.zeros((P, 9), np.float32)
    b_rz = (b_ih + b_hh)[:2 * MEM_DIM]
    for j in range(4):
        biases[:, j] = b_rz[j * P:(j + 1) * P]
    for j in range(2):
        biases[:, 4 + j] = b_hh[2 * MEM_DIM + j * P:2 * MEM_DIM + (j + 1) * P]
        biases[:, 6 + j] = b_ih[2 * MEM_DIM + j * P:2 * MEM_DIM + (j + 1) * P]
    biases[0:FC_DIM, 8] = b_fc

    in_maps = []
    for c in range(N_CORES):
        lo, mid, hi = int(bounds[c]), int(mids[c]), int(bounds[c + 1])
        segs = [(lo, mid, 0), (mid, hi, 1)]
        rel16 = np.zeros(u_tot, np.int16)
        maskv = np.zeros(u_tot, np.float32)
        msgs_u = np.zeros((u_tot, MSG_DIM), np.float32)
        for (a, b, s) in segs:
            n = b - a
            off = s * u_sub
            rel16[off:off + n] = (ids[a:b].astype(np.int64)
                                  - c * R - s * HALF).astype(np.int16)
            maskv[off:off + n] = 1.0
            msgs_u[off:off + n] = msgs[a:b]
        idx_tile = np.tile(
            rel16.reshape(u_tot // 16, 16).T, (8, 1)).astype(np.int16)
        in_maps.append({
            "tab_in": np.ascontiguousarray(tab[c * R:(c + 1) * R]),
            "msgsT": np.ascontiguousarray(msgs_u.T),
            "idx16": np.ascontiguousarray(idx_tile),
            "mask": np.ascontiguousarray(maskv.reshape(n_tiles, P).T),
            "w_ihT": w_ihT, "w_hhT": w_hhT, "w_fcT": w_fcT, "w_outT": w_outT,
            "biases": biases,
        })
    return u_sub, in_maps, (ids, ts)


def kernel(**inputs):
    from concourse.bass_utils import run_bass_kernel_spmd

    u_sub, in_maps, (ids, ts) = _prep(inputs)
    if u_sub not in _CACHE:
        _CACHE[u_sub] = _build(u_sub)
    nc = _CACHE[u_sub]

    trace = bool(int(os.environ.get("KERNEL_TRACE", "0")))
    kw = {}
    if trace:
        kw = dict(trace=True, trace_cores=[0])
    res = run_bass_kernel_spmd(nc, in_maps, core_ids=list(range(N_CORES)), **kw)
    if trace:
        kernel.last_exec_time_ns = res.exec_time_ns
        kernel.last_results = res

    mem = np.empty((N_NODES, MEM_DIM), np.float32)
    for c in range(N_CORES):
        mem[c * R:(c + 1) * R] = res.results[c]["tab_out"]
    lu = np.array(np.asarray(inputs["last_update"], np.float32))
    lu[ids] = ts
    return mem, lu


# revision 16
# speedup vs baseline: 1.0393x; 1.0393x over previous
"""Trainium2 Bass kernel for EnhancedGRU memory updater (scatter_memory).

kernel(**inputs) takes FULL inputs, shards the memory table row-wise over 8
NeuronCores, routes messages to owner shards (ids are sorted so routing is
slicing), and per shard runs on-device:
  - bulk DRAM->DRAM copy of the shard into the output table
  - dma_gather of the active rows (int16 indices relative to 2 sub-shards)
  - GRU cell + 2 linear layers in [gate, u] layout (fp32r matmuls)
  - dma_scatter_add of masked row deltas (new - old) onto the copied table
last_update is assembled host-side (0.4% of total I/O).

Self-contained: hardcodes shapes from the problem spec.
"""

import os
import numpy as np

N_NODES = 500_000
MEM_DIM = 256
MSG_DIM = 172
N_UNIQUE = 131_072
FC_DIM = 64
N_CORES = 8
R = N_NODES // N_CORES          # rows per shard = 62500
HALF = R // 2                    # sub-shard rows (int16-indexable)
P = 128
CHUNK = 512                      # u rows per compute chunk
BLK = 512                        # u rows per gather/scatter DMA op (== CHUNK)

_CACHE = {}


def _build(u_sub):
    import concourse.bass as bass
    import concourse.bacc as bacc
    import concourse.mybir as mybir
    import concourse.tile as tile
    from concourse.bass import _add_dep_helper
    from concourse.masks import make_identity

    f32 = mybir.dt.float32
    f32r = mybir.dt.float32r
    i16 = mybir.dt.int16
    AF = mybir.ActivationFunctionType
    OP = mybir.AluOpType

    u_tot = 2 * u_sub
    n_tiles = u_tot // P
    n_chunks = u_tot // CHUNK
    n_blocks = u_tot // BLK
    blocks_per_sub = u_sub // BLK
    icols = BLK // 16                # idx columns per block

    nc = bacc.Bacc(None, target_bir_lowering=False, debug=False)

    tab_in = nc.dram_tensor("tab_in", (R, MEM_DIM), f32, kind="ExternalInput")
    msgsT = nc.dram_tensor("msgsT", (MSG_DIM, u_tot), f32r, kind="ExternalInput")
    idx16 = nc.dram_tensor("idx16", (P, u_tot // 16), i16, kind="ExternalInput")
    mask = nc.dram_tensor("mask", (P, n_tiles), f32, kind="ExternalInput")
    w_ihT = nc.dram_tensor("w_ihT", (MSG_DIM, 3 * MEM_DIM), f32r, kind="ExternalInput")
    w_hhT = nc.dram_tensor("w_hhT", (MEM_DIM, 3 * MEM_DIM), f32r, kind="ExternalInput")
    w_fcT = nc.dram_tensor("w_fcT", (MEM_DIM, FC_DIM), f32r, kind="ExternalInput")
    w_outT = nc.dram_tensor("w_outT", (FC_DIM + 1, MEM_DIM), f32r, kind="ExternalInput")
    biases = nc.dram_tensor("biases", (P, 9), f32, kind="ExternalInput")

    tab_out = nc.dram_tensor("tab_out", (R, MEM_DIM), f32, kind="ExternalOutput")

    with tile.TileContext(nc) as tc:
        with tc.tile_pool(name="const", bufs=1) as cp, \
             tc.tile_pool(name="store", bufs=1) as sp, \
             tc.tile_pool(name="work", bufs=3) as wp, \
             tc.tile_pool(name="gate", bufs=1) as gp, \
             tc.tile_pool(name="ps", bufs=2, space="PSUM") as pp, \
             tc.tile_pool(name="psrz", bufs=4, space="PSUM") as prz, \
             tc.tile_pool(name="pstail", bufs=2, space="PSUM") as ptl:

            # ---- big shard copy: input -> output, 4 x 16MB DMAs on SP
            # (few big DMAs: Tile allows ~1 in-flight DMA per completion lane,
            # so many small copies serialize ~50us apart)
            flat_in = tab_in[:].rearrange("r d -> (r d)")
            flat_out = tab_out[:].rearrange("r d -> (r d)")
            tot = R * MEM_DIM
            ncopy = 4
            cs = tot // ncopy
            assert cs * ncopy == tot
            copies = []
            for j in range(ncopy):
                copies.append(nc.sync.dma_start(
                    out=flat_out[j * cs:(j + 1) * cs],
                    in_=flat_in[j * cs:(j + 1) * cs]))
            half_copies = [copies[0:2], copies[2:4]]

            # ---- persistent constants (ACT HWDGE queue)
            wih0 = cp.tile([P, 3 * MEM_DIM], f32r, tag="wih0")
            wih1 = cp.tile([MSG_DIM - P, 3 * MEM_DIM], f32r, tag="wih1")
            whh0 = cp.tile([P, 3 * MEM_DIM], f32r, tag="whh0")
            whh1 = cp.tile([P, 3 * MEM_DIM], f32r, tag="whh1")
            wfc0 = cp.tile([P, FC_DIM], f32r, tag="wfc0")
            wfc1 = cp.tile([P, FC_DIM], f32r, tag="wfc1")
            wout = cp.tile([FC_DIM + 1, MEM_DIM], f32r, tag="wout")
            bias_t = cp.tile([P, 9], f32, tag="bias")
            idx_t = cp.tile([P, u_tot // 16], i16, tag="idx")
            mask_t = cp.tile([P, n_tiles], f32, tag="mask")
            ident = cp.tile([P, P], f32, tag="ident")

            nc.scalar.dma_start(out=wih0[:], in_=w_ihT[0:P, :])
            nc.scalar.dma_start(out=wih1[:], in_=w_ihT[P:MSG_DIM, :])
            nc.scalar.dma_start(out=whh0[:], in_=w_hhT[0:P, :])
            nc.scalar.dma_start(out=whh1[:], in_=w_hhT[P:2 * P, :])
            nc.scalar.dma_start(out=wfc0[:], in_=w_fcT[0:P, :])
            nc.scalar.dma_start(out=wfc1[:], in_=w_fcT[P:2 * P, :])
            nc.scalar.dma_start(out=wout[:], in_=w_outT[:])
            nc.scalar.dma_start(out=bias_t[:], in_=biases[:])
            nc.scalar.dma_start(out=idx_t[:], in_=idx16[:])
            nc.scalar.dma_start(out=mask_t[:], in_=mask[:])
            make_identity(nc, ident[:])

            # ---- persistent delta store: [128, n_tiles * 256] f32
            out_store = sp.tile([P, n_tiles * MEM_DIM], f32, tag="ostore")
            os3 = out_store[:].rearrange("p (t d) -> p t d", d=MEM_DIM)

            emitted = set()

            def emit_scatter(b):
                if b in emitted:
                    return
                emitted.add(b)
                sub = b // blocks_per_sub
                sa = nc.gpsimd.dma_scatter_add(
                    out_ap=tab_out[sub * HALF:(sub + 1) * HALF, :],
                    in_ap=os3[:, b * (BLK // P):(b + 1) * (BLK // P), :],
                    idxs_ap=idx_t[:, b * icols:(b + 1) * icols],
                    num_idxs=BLK, num_idxs_reg=BLK, elem_size=MEM_DIM)
                for cpi in half_copies[sub]:
                    _add_dep_helper(sa.ins, cpi.ins, sync=True,
                                    reason="scatter-add after table copy")

            for b in range(n_blocks):
                sub = b // blocks_per_sub
                # interleave sub-0 scatter-adds once sub-0 compute is done
                # (their copy-half should be complete by then; keeps the POOL
                # queue from stalling gathers behind copy waits)
                if b == blocks_per_sub + 2 and not bool(int(
                        os.environ.get("KERNEL_NO_INTERLEAVE", "0"))):
                    for bs in range(blocks_per_sub):
                        emit_scatter(bs)
                # gather BLK h rows (relative to sub-shard): [128, 4, 256]
                g_blk = wp.tile([P, BLK // P, MEM_DIM], f32, tag="g")
                nc.gpsimd.dma_gather(
                    out_ap=g_blk[:],
                    in_ap=tab_in[sub * HALF:(sub + 1) * HALF, :],
                    idxs_ap=idx_t[:, b * icols:(b + 1) * icols],
                    num_idxs=BLK, num_idxs_reg=BLK, elem_size=MEM_DIM)

                if True:
                    c = b
                    u0 = c * CHUNK
                    xT0 = wp.tile([P, CHUNK], f32r, tag="xT0")
                    xT1 = wp.tile([MSG_DIM - P, CHUNK], f32r, tag="xT1")
                    nc.scalar.dma_start(out=xT0[:], in_=msgsT[0:P, u0:u0 + CHUNK])
                    nc.scalar.dma_start(out=xT1[:],
                                        in_=msgsT[P:MSG_DIM, u0:u0 + CHUNK])

                    def hsl(k, g_blk=g_blk):
                        return g_blk[:, k, :]

                    # gi_n = W_ih[512:768] @ xT (x-only; runs before hT ready)
                    gin_ps = []
                    for g2 in range(2):
                        gs = 2 * MEM_DIM + g2 * P
                        t = ptl.tile([P, CHUNK], f32, tag="tail", space="PSUM")
                        gin_ps.append(t)
                        nc.tensor.matmul(out=t[:], lhsT=wih0[:, gs:gs + P],
                                         rhs=xT0[:], start=True, stop=False)
                        nc.tensor.matmul(out=t[:], lhsT=wih1[:, gs:gs + P],
                                         rhs=xT1[:], start=False, stop=True)
                    # r,z: x-side partial sums first (no hT dependency)
                    rz_ps = []
                    for g4 in range(4):
                        gs = g4 * P
                        t = prz.tile([P, CHUNK], f32, tag="rz", space="PSUM")
                        rz_ps.append(t)
                        nc.tensor.matmul(out=t[:], lhsT=wih0[:, gs:gs + P],
                                         rhs=xT0[:], start=True, stop=False)
                        nc.tensor.matmul(out=t[:], lhsT=wih1[:, gs:gs + P],
                                         rhs=xT1[:], start=False, stop=False)
                    # transpose h -> hT (2 x [128, 512]) via PE into PSUM
                    hTp0 = pp.tile([P, CHUNK], f32, tag="pA", space="PSUM")
                    hTp1 = pp.tile([P, CHUNK], f32, tag="pA", space="PSUM")
                    for k in range(4):
                        nc.tensor.transpose(out=hTp0[:, k * P:(k + 1) * P],
                                            in_=hsl(k)[:, 0:P], identity=ident[:])
                        nc.tensor.transpose(out=hTp1[:, k * P:(k + 1) * P],
                                            in_=hsl(k)[:, P:MEM_DIM],
                                            identity=ident[:])
                    hT0 = gp.tile([P, CHUNK], f32r, tag="hT0")
                    hT1 = gp.tile([P, CHUNK], f32r, tag="hT1")
                    nc.vector.tensor_copy(hT0[:], hTp0[:])
                    nc.vector.tensor_copy(hT1[:], hTp1[:])
                    hT_sb = [hT0, hT1]

                    # gh_n = W_hh[512:768] @ hT -> 2 psum tiles [128, 512]
                    ghn_ps = []
                    for g2 in range(2):
                        gs = 2 * MEM_DIM + g2 * P
                        t = pp.tile([P, CHUNK], f32, tag="pA", space="PSUM")
                        ghn_ps.append(t)
                        nc.tensor.matmul(out=t[:], lhsT=whh0[:, gs:gs + P],
                                         rhs=hT0[:], start=True, stop=False)
                        nc.tensor.matmul(out=t[:], lhsT=whh1[:, gs:gs + P],
                                         rhs=hT1[:], start=False, stop=True)
                    # r,z: h-side accumulation
                    for g4 in range(4):
                        gs = g4 * P
                        t = rz_ps[g4]
                        nc.tensor.matmul(out=t[:], lhsT=whh0[:, gs:gs + P],
                                         rhs=hT0[:], start=False, stop=False)
                        nc.tensor.matmul(out=t[:], lhsT=whh1[:, gs:gs + P],
                                         rhs=hT1[:], start=False, stop=True)
                    # ghn + b_hhn -> SBUF (ACT, bias cols 4,5)
                    ghn_sb = []
                    for g2 in range(2):
                        t = gp.tile([P, CHUNK], f32, tag=f"ghnb{g2}")
                        nc.scalar.activation(t[:], ghn_ps[g2][:], AF.Identity,
                                             bias=bias_t[:, 4 + g2:5 + g2])
                        ghn_sb.append(t)
                    # r,z = sigmoid(rz + b) (bias cols 0..3)
                    rz_sb = []
                    for g4 in range(4):
                        t = gp.tile([P, CHUNK], f32, tag=f"rz{g4}")
                        nc.scalar.activation(t[:], rz_ps[g4][:], AF.Sigmoid,
                                             bias=bias_t[:, g4:g4 + 1])
                        rz_sb.append(t)
                    r_sb, z_sb = rz_sb[0:2], rz_sb[2:4]
                    # n = tanh(gin + r*ghn_b + b_ihn) (bias cols 6,7)
                    hn_sb = []
                    for g2 in range(2):
                        tm = gp.tile([P, CHUNK], f32, tag=f"tm{g2}")
                        nc.vector.tensor_tensor(out=tm[:], in0=r_sb[g2][:],
                                                in1=ghn_sb[g2][:], op=OP.mult)
                        nc.vector.tensor_tensor(out=tm[:], in0=tm[:],
                                                in1=gin_ps[g2][:], op=OP.add)
                        n_t = gp.tile([P, CHUNK], f32, tag=f"n{g2}")
                        nc.scalar.activation(n_t[:], tm[:], AF.Tanh,
                                             bias=bias_t[:, 6 + g2:7 + g2])
                        # h_new = n + z*(h - n); d reuses the tm slot,
                        # h_new reuses the hT slot (both dead by then)
                        d_t = gp.tile([P, CHUNK], f32, tag=f"tm{g2}")
                        nc.vector.tensor_tensor(out=d_t[:], in0=hT_sb[g2][:],
                                                in1=n_t[:], op=OP.subtract)
                        nc.vector.tensor_tensor(out=d_t[:], in0=z_sb[g2][:],
                                                in1=d_t[:], op=OP.mult)
                        hn = gp.tile([P, CHUNK], f32r, tag=f"hT{g2}")
                        nc.vector.tensor_tensor(out=hn[:], in0=n_t[:],
                                                in1=d_t[:], op=OP.add)
                        hn_sb.append(hn)
                    # predT = W_fc @ h_new (+b_fc); augmented with a ones row
                    pred_ps = ptl.tile([FC_DIM, CHUNK], f32, tag="tail",
                                       space="PSUM")
                    nc.tensor.matmul(out=pred_ps[:], lhsT=wfc0[:, :],
                                     rhs=hn_sb[0][:], start=True, stop=False)
                    nc.tensor.matmul(out=pred_ps[:], lhsT=wfc1[:, :],
                                     rhs=hn_sb[1][:], start=False, stop=True)
                    predT = gp.tile([FC_DIM + 1, CHUNK], f32r, tag="predT")
                    nc.scalar.activation(predT[0:FC_DIM, :], pred_ps[:],
                                         AF.Identity, bias=bias_t[0:FC_DIM, 8:9])
                    nc.scalar.activation(predT[FC_DIM:FC_DIM + 1, :],
                                         pred_ps[0:1, :], AF.Copy,
                                         bias=1.0, scale=0.0)
                    # out rows (+b_out via ones row), delta = mask*(out - h)
                    for k in range(4):
                        ops = ptl.tile([P, MEM_DIM], f32, tag="tail",
                                       space="PSUM")
                        nc.tensor.matmul(out=ops[:],
                                         lhsT=predT[:, k * P:(k + 1) * P],
                                         rhs=wout[:], start=True, stop=True)
                        t_u = 4 * c + k
                        dl = gp.tile([P, MEM_DIM], f32, tag="dl")
                        nc.vector.tensor_tensor(out=dl[:], in0=ops[:],
                                                in1=hsl(k)[:], op=OP.subtract)
                        nc.vector.tensor_scalar_mul(
                            os3[:, t_u, :], dl[:], mask_t[:, t_u:t_u + 1])



            for b in range(n_blocks):
                emit_scatter(b)

    nc.compile()
    return nc


def _prep(inputs):
    """Host-side shard/routing prep. Returns (u_sub, in_maps, (ids, ts))."""
    ids = np.asarray(inputs["unique_node_ids"])
    msgs = np.asarray(inputs["unique_messages"], dtype=np.float32)
    ts = np.asarray(inputs["timestamps"], dtype=np.float32)
    tab = np.asarray(inputs["memory_table"], dtype=np.float32)

    if not np.all(ids[:-1] <= ids[1:]):
        order = np.argsort(ids, kind="stable")
        ids, msgs, ts = ids[order], msgs[order], ts[order]

    bounds = np.searchsorted(ids, np.arange(N_CORES + 1) * R).astype(np.int64)
    mids = np.searchsorted(ids, np.arange(N_CORES) * R + HALF).astype(np.int64)
    sub_cnt = np.stack([mids - bounds[:-1], bounds[1:] - mids])  # [2, C]
    u_sub = int(np.ceil(max(int(sub_cnt.max()), BLK) / BLK) * BLK)
    u_tot = 2 * u_sub
    n_tiles = u_tot // P

    w_ihT = np.ascontiguousarray(np.asarray(inputs["W_ih"], np.float32).T)
    w_hhT = np.ascontiguousarray(np.asarray(inputs["W_hh"], np.float32).T)
    w_fcT = np.ascontiguousarray(np.asarray(inputs["W_fc"], np.float32).T)
    w_out = np.asarray(inputs["W_out"], np.float32)
    b_out = np.asarray(inputs["b_out"], np.float32)
    w_outT = np.ascontiguousarray(
        np.concatenate([w_out.T, b_out[None, :]], axis=0))  # [65, 256]
    b_ih = np.asarray(inputs["b_ih"], np.float32)
    b_hh = np.asarray(inputs["b_hh"], np.float32)
    b_fc = np.asarray(inputs["b_fc"], np.float32)

    biases = np.zeros((P, 9), np.float32)
    b_rz = (b_ih + b_hh)[:2 * MEM_DIM]
    for j in range(4):
        biases[:, j] = b_rz[j * P:(j + 1) * P]
    for j in range(2):
        biases[:, 4 + j] = b_hh[2 * MEM_DIM + j * P:2 * MEM_DIM + (j + 1) * P]
        biases[:, 6 + j] = b_ih[2 * MEM_DIM + j * P:2 * MEM_DIM + (j + 1) * P]
    biases[0:FC_DIM, 8] = b_fc

    in_maps = []
    for c in range(N_CORES):
        lo, mid, hi = int(bounds[c]), int(mids[c]), int(bounds[c + 1])
        segs = [(lo, mid, 0), (mid, hi, 1)]
        rel16 = np.zeros(u_tot, np.int16)
        maskv = np.zeros(u_tot, np.float32)
        msgs_u = np.zeros((u_tot, MSG_DIM), np.float32)
        for (a, b, s) in segs:
            n = b - a
            off = s * u_sub
            rel16[off:off + n] = (ids[a:b].astype(np.int64)
                                  - c * R - s * HALF).astype(np.int16)
            maskv[off:off + n] = 1.0
            msgs_u[off:off + n] = msgs[a:b]
        idx_tile = np.tile(
            rel16.reshape(u_tot // 16, 16).T, (8, 1)).astype(np.int16)
        in_maps.append({
            "tab_in": np.ascontiguousarray(tab[c * R:(c + 1) * R]),
            "msgsT": np.ascontiguousarray(msgs_u.T),
            "idx16": np.ascontiguousarray(idx_tile),
            "mask": np.ascontiguousarray(maskv.reshape(n_tiles, P).T),
            "w_ihT": w_ihT, "w_hhT": w_hhT, "w_fcT": w_fcT, "w_outT": w_outT,
            "biases": biases,
        })
    return u_sub, in_maps, (ids, ts)


def kernel(**inputs):
    from concourse.bass_utils import run_bass_kernel_spmd

    u_sub, in_maps, (ids, ts) = _prep(inputs)
    if u_sub not in _CACHE:
        _CACHE[u_sub] = _build(u_sub)
    nc = _CACHE[u_sub]

    trace = bool(int(os.environ.get("KERNEL_TRACE", "0")))
    kw = {}
    if trace:
        kw = dict(trace=True, trace_cores=[0])
    res = run_bass_kernel_spmd(nc, in_maps, core_ids=list(range(N_CORES)), **kw)
    if trace:
        kernel.last_exec_time_ns = res.exec_time_ns
        kernel.last_results = res

    mem = np.empty((N_NODES, MEM_DIM), np.float32)
    for c in range(N_CORES):
        mem[c * R:(c + 1) * R] = res.results[c]["tab_out"]
    lu = np.array(np.asarray(inputs["last_update"], np.float32))
    lu[ids] = ts
    return mem, lu
